# revision 29
# baseline (speedup 1.0000x reference)
"""Trainium2 Bass kernel for nn_ModalGenerator (MoE-routed cross-modal generator).

Strategy:
  - seq_len==1 => softmax over one key is identically 1, so attention output is
    just v = tgt @ wv.T + bv. Fold wv/ao_w into one 512x512 matrix per layer
    (host-side), and fold (1-rw) into the output projection.
  - MoE routing: only rows with missing_type==1 need generator 0 (img->text)
    and only missing_type==2 need generator 1 (text->img). Host gathers those
    rows, the device runs the generators on the compacted columns only
    (~1/4 of the batch each), host scatters results back. missing_type==3 rows
    use the (host-computed, tiny) prior MLP; other rows pass through.
  - Data-parallel over 8 NeuronCores: columns sharded, weights replicated.
  - Activations live transposed [H(partitions), cols(free)] in bf16.
  - _MM_MODE selects the heavy-matmul precision for wa/f1/f2: "dr" (fp8e4m3
    DoubleRow), "fp8" (plain fp8), "bf16". io/op projections and the mean
    statistic always run bf16. Host-side validation vs the fp64 reference
    gives ~1e-2 rel err for fp8 modes, ~2e-3 for bf16 (budget 2e-2).
  - LayerNorm stats via scaled-ones matmuls, rsqrt via fp32 bit-hack on DVE
    + one Newton step on the otherwise-idle GPSIMD (Pool) engine.
  - The two generators are interleaved at sub-step granularity (skew) so one
    generator's matmuls hide the other's LayerNorm dependency chain.
"""

import math

import numpy as np
import ml_dtypes

import concourse.bacc as bacc
import concourse.mybir as mybir
import concourse.tile as tile
from concourse.bass_utils import run_bass_kernel_spmd

f32 = mybir.dt.float32
bf16 = mybir.dt.bfloat16
fp8 = mybir.dt.float8e4
i32 = mybir.dt.int32
AF = mybir.ActivationFunctionType
ALU = mybir.AluOpType
DR = mybir.MatmulPerfMode.DoubleRow

_MM_MODE = "dr"           # "dp" | "dr" | "fp8" | "bf16"
_SKEW = 2
_CAST_ACT = False
_YSQ_ACT = False
_X8_DVE = False
_H_BUFS = 1
_DMA_SPLIT = False
_SUBS_EARLY = False
_IN_BUFS = 1
_DRAIN_ACT = False
_MM_BUFS = 3
_MM_ASYM = False

NP_BF16 = ml_dtypes.bfloat16
NP_FP8 = ml_dtypes.float8_e4m3

H = 512
L = 3
NH = 8
N_CORES = 8
KC = H // 128             # 4 K-chunks of the hidden dim
FH = 4 * H                # 2048 FFN hidden
FKC = FH // 128           # 16
LN_EPS = 1e-5
MAGIC = 0x5F3759DF
ONES_S = 2.0 ** -9        # 1/H, exact in bf16
ONES_Q = 2.0 ** -6        # exact (normal) in fp8e4m3
SQ_SCALE = 2.0 ** -1.5    # Square pre-scale so that ONES_Q * sum(sq) = E[y^2]

# param pack column layout (per generator): [128, 128] f32
_P_IPB = 0
_P_LAYER = 4              # + 40*l: ba 0..3 | f1b 4..19 | f2b 20..23
#                                  | ln1g 24..27 | ln1b 28..31 | ln2g 32..35 | ln2b 36..39
_P_OPB = 124


def _pack_pcol(vec):
    """[n*128] vector -> [128, n] chunk-column layout."""
    return np.ascontiguousarray(np.asarray(vec, np.float32).reshape(-1, 128).T)


def _sb_pack(wT):
    """[K, M] (K mult of 128) -> [128, (K/128)*M] SBUF chunk-major layout."""
    K, M = wT.shape
    return np.ascontiguousarray(
        wT.reshape(K // 128, 128, M).transpose(1, 0, 2).reshape(128, -1))


def _sb_pack_dr(wT):
    """[K, M] -> [128, (K/256)*(M/128)*256] DR-native layout: per partition
    [k2][m][i in 2][128 cols], so each DoubleRow lhsT slice is contiguous."""
    K, M = wT.shape
    A = wT.reshape(K // 256, 2, 128, M // 128, 128)
    return np.ascontiguousarray(A.transpose(2, 0, 3, 1, 4).reshape(128, -1))


def _ntiles(C):
    if C <= 512:
        return [(0, C)]
    h = ((C // 2) + 31) // 32 * 32
    return [(0, h), (h, C)]


def _build_program(C0, C1, skip_b, repeat=1, mode=None):
    mode = mode or _MM_MODE
    wdt = bf16 if mode == "bf16" else fp8
    DP = mybir.MatmulPerfMode.DoublePixel

    nc = bacc.Bacc("TRN2", target_bir_lowering=False, debug=False,
                   num_devices=N_CORES)

    dram = {}
    for g, C in ((0, C0), (1, C1)):
        dram[f"src{g}"] = nc.dram_tensor(f"src{g}", [128, KC * C], bf16, kind="ExternalInput")
        dram[f"tgt{g}"] = nc.dram_tensor(f"tgt{g}", [128, KC * C], wdt, kind="ExternalInput")
        dram[f"io{g}"] = nc.dram_tensor(f"io{g}", [128, KC * H], bf16, kind="ExternalInput")
        dram[f"op{g}"] = nc.dram_tensor(f"op{g}", [128, KC * H], bf16, kind="ExternalInput")
        dram[f"wa{g}"] = nc.dram_tensor(f"wa{g}", [L, 128, KC * H], wdt, kind="ExternalInput")
        dram[f"f1{g}"] = nc.dram_tensor(f"f1{g}", [L, 128, KC * FH], wdt, kind="ExternalInput")
        dram[f"f2{g}"] = nc.dram_tensor(f"f2{g}", [L, 128, FKC * H], wdt, kind="ExternalInput")
        dram[f"par{g}"] = nc.dram_tensor(f"par{g}", [128, 128], f32, kind="ExternalInput")
        dram[f"out{g}"] = nc.dram_tensor(f"out{g}", [128, KC * C], f32, kind="ExternalOutput")
    dram["ones_s"] = nc.dram_tensor("ones_s", [128, 128], bf16, kind="ExternalInput")
    dram["ones_q"] = nc.dram_tensor("ones_q", [128, 2, 128], fp8, kind="ExternalInput")

    with tile.TileContext(nc) as tc:
        with (
            tc.tile_pool(name="sb", bufs=2) as sb,
            tc.tile_pool(name="ps", bufs=4, space="PSUM") as psp,
        ):
            ones_s = sb.tile([128, 128], bf16, tag="ones_s", bufs=1)
            nc.sync.dma_start(ones_s[:], dram["ones_s"].ap())
            ones_q = sb.tile([128, 2, 128], fp8, tag="ones_q", bufs=1)
            nc.sync.dma_start(ones_q[:], dram["ones_q"].ap())

            def contract(ps, w, rhs, m, nk):
                """ps += w-chunks.T @ rhs-chunks over nk k-chunks of 128.
                w: weight tile (layout depends on mode), m: output block index.
                rhs: [128, nk, C] tile slice."""
                if mode == "dr":
                    for k in range(nk // 2):
                        nc.tensor.matmul(
                            ps[:], w[:, k, m, :, :],
                            rhs[:, 2 * k:2 * k + 2, :],
                            start=(k == 0), stop=(k == nk // 2 - 1), perf_mode=DR)
                else:
                    pm = DP if mode == "dp" else None
                    for k in range(nk):
                        nc.tensor.matmul(
                            ps[:], w[:, k, 128 * m:128 * m + 128], rhs[:, k, :],
                            start=(k == 0), stop=(k == nk - 1), perf_mode=pm)

            def emit_ln(g, C, tiles, y, xn, par, gcol, bcol, skip_beta, skip_gamma, x8):
                """y [128,KC,C] bf16 -> xn = LN(y)*g+b bf16; x8 = fp8(xn) if given."""
                # qscale: q_ps = qscale * E[y^2]; sqrt(qscale) folded into Newton.
                ysq_act = _YSQ_ACT and mode != "bf16"
                qscale = 1.0 if (mode == "bf16" or ysq_act) else 8.0
                rq = math.sqrt(qscale)
                for c0, c1 in tiles:
                    Ct = c1 - c0
                    # mean: (2^-9 ones) @ y, accumulated over KC chunks
                    s_ps = psp.tile([128, Ct], f32, tag=f"st{g}", bufs=1)
                    for k in range(KC):
                        nc.tensor.matmul(s_ps[:], ones_s[:], y[:, k, c0:c1],
                                         start=(k == 0), stop=(k == KC - 1))
                    m_bc = sb.tile([128, Ct], bf16, tag=f"m{g}", bufs=1)
                    nc.vector.tensor_copy(m_bc[:], s_ps[:])
                    msq = sb.tile([128, Ct], bf16, tag=f"msq{g}", bufs=1)
                    nc.gpsimd.tensor_mul(msq[:], m_bc[:], m_bc[:])
                    # E[y^2]: ysq = y*y (Pool), q = scaled-ones @ ysq
                    ysq = sb.tile([128, KC, Ct], fp8 if mode != "bf16" else bf16,
                                  tag=f"ysq{g}", bufs=1)
                    if ysq_act:
                        for k in range(KC):
                            nc.scalar.activation(ysq[:, k, :], y[:, k, c0:c1],
                                                 AF.Square, scale=SQ_SCALE)
                    else:
                        for k in range(KC):
                            nc.gpsimd.tensor_mul(ysq[:, k, :], y[:, k, c0:c1],
                                                 y[:, k, c0:c1])
                    u1s = []
                    def emit_subs():
                        for m in range(KC):
                            u1 = sb.tile([128, Ct], bf16, tag=f"u1{g}", bufs=4)
                            nc.vector.tensor_sub(u1[:], y[:, m, c0:c1], m_bc[:])
                            u1s.append(u1)
                    if _SUBS_EARLY:
                        emit_subs()
                    q_ps = psp.tile([128, Ct], f32, tag=f"st{g}", bufs=1)
                    if mode == "dr":
                        for k in range(KC // 2):
                            nc.tensor.matmul(q_ps[:], ones_q[:],
                                             ysq[:, 2 * k:2 * k + 2, :],
                                             start=(k == 0), stop=(k == KC // 2 - 1),
                                             perf_mode=DR)
                    else:
                        ones_for_q = ones_s[:] if mode == "bf16" else ones_q[:, 0, :]
                        qpm = DP if mode == "dp" else None
                        for k in range(KC):
                            nc.tensor.matmul(q_ps[:], ones_for_q, ysq[:, k, :],
                                             start=(k == 0), stop=(k == KC - 1),
                                             perf_mode=qpm)
                    msqe = sb.tile([128, Ct], bf16, tag=f"msqe{g}", bufs=1)
                    nc.vector.tensor_scalar(msqe[:], msq[:], qscale, qscale * LN_EPS,
                                            ALU.mult, ALU.subtract)
                    z = sb.tile([128, Ct], f32, tag=f"z{g}", bufs=1)
                    nc.vector.tensor_sub(z[:], q_ps[:], msqe[:])
                    # rsqrt(z/qscale): bit-hack (DVE) + 1 Newton step (GPSIMD),
                    # sqrt(qscale) folded into the Newton constants.
                    ti = sb.tile([128, Ct], i32, tag=f"ch{g}", bufs=2)
                    nc.vector.tensor_scalar(ti[:], z[:].bitcast(i32), 1, None,
                                            ALU.arith_shift_right)
                    r0 = sb.tile([128, Ct], f32, tag=f"r0{g}", bufs=1)
                    nc.vector.tensor_scalar(r0[:].bitcast(i32), ti[:], -1, MAGIC,
                                            ALU.mult, ALU.add)
                    u = sb.tile([128, Ct], f32, tag=f"ch{g}", bufs=2)
                    nc.gpsimd.tensor_mul(u[:], r0[:], r0[:])
                    v = sb.tile([128, Ct], f32, tag=f"ch{g}", bufs=2)
                    nc.gpsimd.tensor_mul(v[:], u[:], z[:])
                    w = sb.tile([128, Ct], f32, tag=f"ch{g}", bufs=2)
                    nc.vector.tensor_scalar(w[:], v[:], -0.5 * rq, 1.5 * rq,
                                            ALU.mult, ALU.add)
                    if not _SUBS_EARLY:
                        emit_subs()
                    rstd = sb.tile([128, Ct], bf16, tag=f"rstd{g}", bufs=1)
                    nc.gpsimd.tensor_mul(rstd[:], w[:], r0[:])
                    for m in range(KC):
                        u1 = u1s[m]
                        if skip_gamma and skip_beta:
                            nc.vector.tensor_mul(xn[:, m, c0:c1], u1[:], rstd[:])
                            if x8 is not None:
                                if _X8_DVE:
                                    nc.vector.tensor_copy(x8[:, m, c0:c1], xn[:, m, c0:c1])
                                else:
                                    nc.gpsimd.tensor_mul(x8[:, m, c0:c1], u1[:], rstd[:])
                        elif skip_beta:
                            nc.vector.scalar_tensor_tensor(
                                xn[:, m, c0:c1], u1[:], par[:, gcol + m:gcol + m + 1],
                                rstd[:], ALU.mult, ALU.mult)
                            if x8 is not None:
                                nc.scalar.activation(x8[:, m, c0:c1], xn[:, m, c0:c1],
                                                     AF.Copy)
                        else:
                            u2 = sb.tile([128, Ct], bf16, tag=f"u2{g}", bufs=1)
                            nc.vector.scalar_tensor_tensor(
                                u2[:], u1[:], par[:, gcol + m:gcol + m + 1],
                                rstd[:], ALU.mult, ALU.mult)
                            nc.vector.tensor_scalar(
                                xn[:, m, c0:c1], u2[:], par[:, bcol + m:bcol + m + 1],
                                None, ALU.add)
                            if x8 is not None:
                                nc.scalar.activation(x8[:, m, c0:c1], xn[:, m, c0:c1],
                                                     AF.Copy)

            def step_init(st):
                g, C = st["g"], st["C"]
                st["src"] = sb.tile([128, KC, C], bf16, tag=f"src{g}", bufs=_IN_BUFS, name=f"src{g}")
                nc.sync.dma_start(st["src"][:], dram[f"src{g}"].ap())
                st["par"] = sb.tile([128, 128], f32, tag=f"par{g}", bufs=1, name=f"parw{g}")
                nc.sync.dma_start(st["par"][:], dram[f"par{g}"].ap())
                st["io"] = sb.tile([128, KC * H], bf16, tag=f"io{g}", bufs=1, name=f"iow{g}")
                nc.sync.dma_start(st["io"][:], dram[f"io{g}"].ap())
                st["tgt"] = sb.tile([128, KC, C], wdt, tag=f"tgt{g}", bufs=_IN_BUFS, name=f"tgt{g}")
                nc.sync.dma_start(st["tgt"][:], dram[f"tgt{g}"].ap())
                st["op"] = sb.tile([128, KC * H], bf16, tag=f"op{g}", bufs=1, name=f"opw{g}")
                nc.sync.dma_start(st["op"][:], dram[f"op{g}"].ap())

            def step_io(st):
                g, C, par = st["g"], st["C"], st["par"]
                x = sb.tile([128, KC, C], bf16, tag=f"x{g}", bufs=2, name=f"x{g}")
                for c0, c1 in st["tiles"]:
                    for m in range(KC):
                        ps = psp.tile([128, c1 - c0], f32, tag=f"mm{g}", bufs=(_MM_BUFS if g == 0 else 6 - _MM_BUFS) if _MM_ASYM else _MM_BUFS, name=f"ps{g}")
                        for k in range(KC):
                            nc.tensor.matmul(
                                ps[:], st["io"][:, k * H + 128 * m:k * H + 128 * (m + 1)],
                                st["src"][:, k, c0:c1],
                                start=(k == 0), stop=(k == KC - 1))
                        nc.scalar.activation(x[:, m, c0:c1], ps[:], AF.Identity,
                                             bias=par[:, _P_IPB + m:_P_IPB + m + 1])
                st["x"] = x

            def step_attn(st, l):
                g, C, par, x = st["g"], st["C"], st["par"], st["x"]
                wbufs = 1 if mode == "bf16" else 2
                if mode == "dr":
                    st["wa"] = sb.tile([128, KC // 2, H // 128, 2, 128], wdt, tag=f"wa{g}", bufs=2, name=f"waw{g}")
                    (nc.gpsimd if _DMA_SPLIT else nc.sync).dma_start(st["wa"][:], dram[f"wa{g}"].ap()[l])
                    st["f1w"] = sb.tile([128, KC // 2, FH // 128, 2, 128], wdt, tag=f"f1{g}", bufs=wbufs, name=f"f1w{g}")
                    nc.sync.dma_start(st["f1w"][:], dram[f"f1{g}"].ap()[l])
                    st["f2w"] = sb.tile([128, FKC // 2, H // 128, 2, 128], wdt, tag=f"f2{g}", bufs=wbufs, name=f"f2w{g}")
                    nc.sync.dma_start(st["f2w"][:], dram[f"f2{g}"].ap()[l])
                else:
                    st["wa"] = sb.tile([128, KC, H], wdt, tag=f"wa{g}", bufs=2, name=f"waw{g}")
                    nc.sync.dma_start(st["wa"][:], dram[f"wa{g}"].ap()[l])
                    st["f1w"] = sb.tile([128, KC, FH], wdt, tag=f"f1{g}", bufs=wbufs, name=f"f1w{g}")
                    nc.sync.dma_start(st["f1w"][:], dram[f"f1{g}"].ap()[l])
                    st["f2w"] = sb.tile([128, FKC, H], wdt, tag=f"f2{g}", bufs=wbufs, name=f"f2w{g}")
                    nc.sync.dma_start(st["f2w"][:], dram[f"f2{g}"].ap()[l])
                pb = _P_LAYER + 40 * l
                y = sb.tile([128, KC, C], bf16, tag=f"y{g}", bufs=1, name=f"y{g}")
                for c0, c1 in st["tiles"]:
                    for m in range(KC):
                        ps = psp.tile([128, c1 - c0], f32, tag=f"mm{g}", bufs=(_MM_BUFS if g == 0 else 6 - _MM_BUFS) if _MM_ASYM else _MM_BUFS, name=f"ps{g}")
                        contract(ps, st["wa"], st["tgt"][:, :, c0:c1], m, KC)
                        nc.vector.scalar_tensor_tensor(
                            y[:, m, c0:c1], ps[:],
                            par[:, pb + m:pb + m + 1], x[:, m, c0:c1],
                            ALU.add, ALU.add)
                st["y"] = y

            def step_ln1(st, l):
                g, C = st["g"], st["C"]
                pb = _P_LAYER + 40 * l
                xn = sb.tile([128, KC, C], bf16, tag=f"x{g}", bufs=2, name=f"x{g}")
                if mode == "bf16":
                    x8 = None
                else:
                    x8 = sb.tile([128, KC, C], fp8, tag=f"x8{g}", bufs=1, name=f"x8{g}")
                emit_ln(g, C, st["tiles"], st["y"], xn, st["par"],
                        pb + 24, pb + 28, skip_b[g][0], skip_b[g][2], x8)
                st["x"] = xn
                st["x8"] = x8 if x8 is not None else xn

            def step_f1(st, l):
                g, C, par = st["g"], st["C"], st["par"]
                pb = _P_LAYER + 40 * l
                hh = sb.tile([128, FKC, C], fp8 if mode != "bf16" else bf16,
                             tag=f"h{g}", bufs=_H_BUFS, name=f"h{g}")
                for c0, c1 in st["tiles"]:
                    for m in range(FKC):
                        ps = psp.tile([128, c1 - c0], f32, tag=f"mm{g}", bufs=(_MM_BUFS if g == 0 else 6 - _MM_BUFS) if _MM_ASYM else _MM_BUFS, name=f"ps{g}")
                        contract(ps, st["f1w"], st["x8"][:, :, c0:c1], m, KC)
                        nc.scalar.activation(hh[:, m, c0:c1], ps[:], AF.Gelu,
                                             bias=par[:, pb + 4 + m:pb + 4 + m + 1])
                st["h"] = hh

            def step_f2(st, l):
                g, C, par = st["g"], st["C"], st["par"]
                pb = _P_LAYER + 40 * l
                y2 = sb.tile([128, KC, C], bf16, tag=f"y{g}", bufs=1, name=f"y{g}")
                for c0, c1 in st["tiles"]:
                    for m in range(KC):
                        ps = psp.tile([128, c1 - c0], f32, tag=f"mm{g}", bufs=(_MM_BUFS if g == 0 else 6 - _MM_BUFS) if _MM_ASYM else _MM_BUFS, name=f"ps{g}")
                        contract(ps, st["f2w"], st["h"][:, :, c0:c1], m, FKC)
                        if _DRAIN_ACT:
                            t2 = sb.tile([128, c1 - c0], bf16, tag=f"t2{g}", bufs=2, name=f"t2{g}")
                            nc.scalar.activation(t2[:], ps[:], AF.Identity,
                                                 bias=par[:, pb + 20 + m:pb + 20 + m + 1])
                            nc.vector.tensor_add(y2[:, m, c0:c1], t2[:],
                                                 st["x"][:, m, c0:c1])
                        else:
                            nc.vector.scalar_tensor_tensor(
                                y2[:, m, c0:c1], ps[:],
                                par[:, pb + 20 + m:pb + 20 + m + 1],
                                st["x"][:, m, c0:c1], ALU.add, ALU.add)
                st["y"] = y2

            def step_ln2(st, l):
                g, C = st["g"], st["C"]
                pb = _P_LAYER + 40 * l
                xn2 = sb.tile([128, KC, C], bf16, tag=f"x{g}", bufs=2, name=f"x{g}")
                emit_ln(g, C, st["tiles"], st["y"], xn2, st["par"],
                        pb + 32, pb + 36, skip_b[g][1], skip_b[g][3], None)
                st["x"] = xn2

            def step_op(st):
                g, C, par = st["g"], st["C"], st["par"]
                for c0, c1 in st["tiles"]:
                    for m in range(KC):
                        ps = psp.tile([128, c1 - c0], f32, tag=f"mm{g}", bufs=(_MM_BUFS if g == 0 else 6 - _MM_BUFS) if _MM_ASYM else _MM_BUFS, name=f"ps{g}")
                        for k in range(KC):
                            nc.tensor.matmul(
                                ps[:],
                                st["op"][:, k * H + 128 * m:k * H + 128 * (m + 1)],
                                st["x"][:, k, c0:c1],
                                start=(k == 0), stop=(k == KC - 1))
                        ot = sb.tile([128, c1 - c0], f32, tag=f"o{g}", bufs=_IN_BUFS, name=f"ot{g}")
                        nc.scalar.activation(ot[:], ps[:], AF.Identity,
                                             bias=par[:, _P_OPB + m:_P_OPB + m + 1])
                        nc.sync.dma_start(
                            dram[f"out{g}"].ap()[:, m * C + c0:m * C + c1], ot[:])

            for _rep in range(repeat):
                sts = [{"g": g, "C": C, "tiles": _ntiles(C)}
                       for g, C in ((0, C0), (1, C1))]

                def steps_for(st):
                    seq = [lambda st=st: (step_init(st), step_io(st))]
                    for l in range(L):
                        seq.append(lambda st=st, l=l: step_attn(st, l))
                        seq.append(lambda st=st, l=l: step_ln1(st, l))
                        seq.append(lambda st=st, l=l: step_f1(st, l))
                        seq.append(lambda st=st, l=l: step_f2(st, l))
                        seq.append(lambda st=st, l=l: step_ln2(st, l))
                    seq.append(lambda st=st: step_op(st))
                    return seq

                s0, s1 = steps_for(sts[0]), steps_for(sts[1])
                for i in range(len(s0) + _SKEW):
                    if i < len(s0):
                        s0[i]()
                    if 0 <= i - _SKEW < len(s1):
                        s1[i - _SKEW]()

    nc.compile()
    return nc


_CACHE = {}


def _get_program(C0, C1, skip_b, repeat=1):
    key = (C0, C1, skip_b, repeat, _MM_MODE, _SKEW, _CAST_ACT,
           _YSQ_ACT, _X8_DVE, _H_BUFS, _DMA_SPLIT, _SUBS_EARLY, _IN_BUFS,
           _DRAIN_ACT, _MM_BUFS, _MM_ASYM)
    if key not in _CACHE:
        _CACHE[key] = _build_program(C0, C1, skip_b, repeat)
    return _CACHE[key]


def _np_wdt():
    return NP_BF16 if _MM_MODE == "bf16" else NP_FP8


def _prep_gen_weights(i, g_ipw, g_ipb, g_qkv_w, g_qkv_b, g_ao_w, g_ao_b,
                      g_ln1g, g_ln1b, g_ln2g, g_ln2b, g_f1w, g_f1b, g_f2w,
                      g_f2b, g_opw, g_opb, g_rw):
    np_wdt = _np_wdt()
    wa, ba = [], []
    for l in range(L):
        _wq, _wk, wv = np.split(g_qkv_w[i, l], 3, axis=0)
        _bq, _bk, bv = np.split(g_qkv_b[i, l], 3)
        wa.append((g_ao_w[i, l] @ wv).T)                 # [K=H, M=H]
        ba.append(g_ao_b[i, l] + bv @ g_ao_w[i, l].T)
    rw = float(g_rw[i])
    io = _sb_pack(g_ipw[i].T).astype(NP_BF16)
    op = _sb_pack((1.0 - rw) * g_opw[i].T).astype(NP_BF16)
    packw = _sb_pack_dr if _MM_MODE == "dr" else _sb_pack
    waP = np.stack([packw(wa[l]).astype(np_wdt) for l in range(L)])
    f1P = np.stack([packw(g_f1w[i, l].T).astype(np_wdt) for l in range(L)])
    f2P = np.stack([packw(g_f2w[i, l].T).astype(np_wdt) for l in range(L)])

    par = np.zeros((128, 128), np.float32)
    par[:, _P_IPB:_P_IPB + KC] = _pack_pcol(g_ipb[i])
    for l in range(L):
        pb = _P_LAYER + 40 * l
        par[:, pb:pb + 4] = _pack_pcol(ba[l])
        par[:, pb + 4:pb + 20] = _pack_pcol(g_f1b[i, l])
        par[:, pb + 20:pb + 24] = _pack_pcol(g_f2b[i, l])
        par[:, pb + 24:pb + 28] = _pack_pcol(g_ln1g[i, l])
        par[:, pb + 28:pb + 32] = _pack_pcol(g_ln1b[i, l])
        par[:, pb + 32:pb + 36] = _pack_pcol(g_ln2g[i, l])
        par[:, pb + 36:pb + 40] = _pack_pcol(g_ln2b[i, l])
    par[:, _P_OPB:_P_OPB + KC] = _pack_pcol((1.0 - rw) * g_opb[i])

    skip = (bool(np.all(g_ln1b[i] == 0.0)), bool(np.all(g_ln2b[i] == 0.0)),
            bool(np.all(g_ln1g[i] == 1.0)), bool(np.all(g_ln2g[i] == 1.0)))
    return {"io": io, "op": op, "wa": waP, "f1": f1P, "f2": f2P, "par": par}, skip, rw


def _prepare(inputs, repeat=1):
    """Host-side prep. Returns (nc, in_maps, assemble) where assemble(results)
    builds the final outputs."""
    image = np.asarray(inputs["image_features"], np.float32)
    text = np.asarray(inputs["text_features"], np.float32)
    mt = np.asarray(inputs["missing_type"])

    idx1 = np.nonzero(mt == 1)[0]      # gen0 (img -> text) fills text
    idx2 = np.nonzero(mt == 2)[0]      # gen1 (text -> img) fills img
    idx3 = np.nonzero(mt == 3)[0]

    gw = {k: np.asarray(v) for k, v in inputs.items() if k.startswith("g_")}
    w0, skip0, rw0 = _prep_gen_weights(0, **gw)
    w1, skip1, rw1 = _prep_gen_weights(1, **gw)

    # prior MLP on host (tiny)
    pe = np.asarray(inputs["prior_emb"], np.float64)
    t = pe @ np.asarray(inputs["prior_w1"], np.float64).T + np.asarray(inputs["prior_b1"], np.float64)
    t = 0.5 * t * (1.0 + np.vectorize(math.erf)(t / math.sqrt(2.0)))
    prior = (t @ np.asarray(inputs["prior_w2"], np.float64).T
             + np.asarray(inputs["prior_b2"], np.float64)).astype(np.float32)
    p_img, p_text = prior[0, :H], prior[0, H:]

    imgT = np.ascontiguousarray(image.T)
    textT = np.ascontiguousarray(text.T)
    np_wdt = _np_wdt()

    def shard_cols(Tsrc, Ttgt, idx):
        n_pc = max(1, -(-len(idx) // N_CORES))
        C = max(256, -(-n_pc // 32) * 32)
        pad = np.zeros(N_CORES * C, np.int64)
        pad[:len(idx)] = idx
        pad = pad.reshape(N_CORES, C)
        return C, [_sb_pack(Tsrc[:, pad[c]]).astype(NP_BF16) for c in range(N_CORES)], \
            [_sb_pack(Ttgt[:, pad[c]]).astype(np_wdt) for c in range(N_CORES)]

    C0, src0, tgt0 = shard_cols(imgT, textT, idx1)
    C1, src1, tgt1 = shard_cols(textT, imgT, idx2)

    nc = _get_program(C0, C1, (skip0, skip1), repeat)

    ones_s = np.full((128, 128), ONES_S, NP_BF16)
    ones_q = np.full((128, 2, 128), ONES_Q, NP_FP8)
    in_maps = []
    for c in range(N_CORES):
        in_maps.append({
            "src0": src0[c], "tgt0": tgt0[c], "src1": src1[c], "tgt1": tgt1[c],
            "io0": w0["io"], "op0": w0["op"], "wa0": w0["wa"], "f10": w0["f1"],
            "f20": w0["f2"], "par0": w0["par"],
            "io1": w1["io"], "op1": w1["op"], "wa1": w1["wa"], "f11": w1["f1"],
            "f21": w1["f2"], "par1": w1["par"],
            "ones_s": ones_s, "ones_q": ones_q,
        })

    def assemble(results):
        def gather_out(name, C, idx, rw, full):
            cols = [results[c][name].reshape(128, KC, C).transpose(1, 0, 2).reshape(H, C)
                    for c in range(N_CORES)]
            allc = np.concatenate(cols, axis=1)[:, :len(idx)]
            return rw * full[idx] + allc.T

        enhanced_text = text.copy()
        if len(idx1):
            enhanced_text[idx1] = gather_out("out0", C0, idx1, rw0, text)
        enhanced_img = image.copy()
        if len(idx2):
            enhanced_img[idx2] = gather_out("out1", C1, idx2, rw1, image)
        if len(idx3):
            enhanced_img[idx3] = p_img
            enhanced_text[idx3] = p_text
        return enhanced_img, enhanced_text

    return nc, in_maps, assemble


def kernel(**inputs):
    nc, in_maps, assemble = _prepare(inputs)
    res = run_bass_kernel_spmd(nc, in_maps, list(range(N_CORES)))
    return assemble(res.results)


# revision 30
# speedup vs baseline: 1.0790x; 1.0790x over previous
"""Trainium2 Bass kernel for nn_ModalGenerator (MoE-routed cross-modal generator).

Strategy:
  - seq_len==1 => softmax over one key is identically 1, so attention output is
    just v = tgt @ wv.T + bv. Fold wv/ao_w into one 512x512 matrix per layer
    (host-side), and fold (1-rw) into the output projection.
  - MoE routing: only rows with missing_type==1 need generator 0 (img->text)
    and only missing_type==2 need generator 1 (text->img). Host gathers those
    rows, the device runs the generators on the compacted columns only
    (~1/4 of the batch each), host scatters results back. missing_type==3 rows
    use the (host-computed, tiny) prior MLP; other rows pass through.
  - Data-parallel over 8 NeuronCores: columns sharded, weights replicated.
  - Activations live transposed [H(partitions), cols(free)] in bf16.
  - _MM_MODE selects the heavy-matmul precision for wa/f1/f2: "dr" (fp8e4m3
    DoubleRow), "fp8" (plain fp8), "bf16". io/op projections and the mean
    statistic always run bf16. Host-side validation vs the fp64 reference
    gives ~1e-2 rel err for fp8 modes, ~2e-3 for bf16 (budget 2e-2).
  - LayerNorm stats via scaled-ones matmuls, rsqrt via fp32 bit-hack on DVE
    + one Newton step on the otherwise-idle GPSIMD (Pool) engine.
  - The two generators are interleaved at sub-step granularity (skew) so one
    generator's matmuls hide the other's LayerNorm dependency chain.
"""

import math

import numpy as np
import ml_dtypes

import concourse.bacc as bacc
import concourse.mybir as mybir
import concourse.tile as tile
from concourse.bass_utils import run_bass_kernel_spmd

f32 = mybir.dt.float32
bf16 = mybir.dt.bfloat16
fp8 = mybir.dt.float8e4
i32 = mybir.dt.int32
AF = mybir.ActivationFunctionType
ALU = mybir.AluOpType
DR = mybir.MatmulPerfMode.DoubleRow

_MM_MODE = "dr"           # "dp" | "dr" | "fp8" | "bf16"
_SKEW = 2
_CAST_ACT = False
_YSQ_ACT = False
_X8_DVE = False
_H_BUFS = 1
_DMA_SPLIT = False
_SUBS_EARLY = False
_IN_BUFS = 1
_DRAIN_ACT = False
_MM_BUFS = 3
_MM_ASYM = False
_X8_SPLIT = False

NP_BF16 = ml_dtypes.bfloat16
NP_FP8 = ml_dtypes.float8_e4m3

H = 512
L = 3
NH = 8
N_CORES = 8
KC = H // 128             # 4 K-chunks of the hidden dim
FH = 4 * H                # 2048 FFN hidden
FKC = FH // 128           # 16
LN_EPS = 1e-5
MAGIC = 0x5F3759DF
ONES_S = 2.0 ** -9        # 1/H, exact in bf16
ONES_Q = 2.0 ** -6        # exact (normal) in fp8e4m3
SQ_SCALE = 2.0 ** -1.5    # Square pre-scale so that ONES_Q * sum(sq) = E[y^2]

# param pack column layout (per generator): [128, 128] f32
_P_IPB = 0
_P_LAYER = 4              # + 40*l: ba 0..3 | f1b 4..19 | f2b 20..23
#                                  | ln1g 24..27 | ln1b 28..31 | ln2g 32..35 | ln2b 36..39
_P_OPB = 124


def _pack_pcol(vec):
    """[n*128] vector -> [128, n] chunk-column layout."""
    return np.ascontiguousarray(np.asarray(vec, np.float32).reshape(-1, 128).T)


def _sb_pack(wT):
    """[K, M] (K mult of 128) -> [128, (K/128)*M] SBUF chunk-major layout."""
    K, M = wT.shape
    return np.ascontiguousarray(
        wT.reshape(K // 128, 128, M).transpose(1, 0, 2).reshape(128, -1))


def _sb_pack_dr(wT):
    """[K, M] -> [128, (K/256)*(M/128)*256] DR-native layout: per partition
    [k2][m][i in 2][128 cols], so each DoubleRow lhsT slice is contiguous."""
    K, M = wT.shape
    A = wT.reshape(K // 256, 2, 128, M // 128, 128)
    return np.ascontiguousarray(A.transpose(2, 0, 3, 1, 4).reshape(128, -1))


def _ntiles(C):
    if C <= 512:
        return [(0, C)]
    h = ((C // 2) + 31) // 32 * 32
    return [(0, h), (h, C)]


def _build_program(C0, C1, skip_b, repeat=1, mode=None):
    mode = mode or _MM_MODE
    wdt = bf16 if mode == "bf16" else fp8
    DP = mybir.MatmulPerfMode.DoublePixel

    nc = bacc.Bacc("TRN2", target_bir_lowering=False, debug=False,
                   num_devices=N_CORES)

    dram = {}
    for g, C in ((0, C0), (1, C1)):
        dram[f"src{g}"] = nc.dram_tensor(f"src{g}", [128, KC * C], bf16, kind="ExternalInput")
        dram[f"tgt{g}"] = nc.dram_tensor(f"tgt{g}", [128, KC * C], wdt, kind="ExternalInput")
        dram[f"io{g}"] = nc.dram_tensor(f"io{g}", [128, KC * H], bf16, kind="ExternalInput")
        dram[f"op{g}"] = nc.dram_tensor(f"op{g}", [128, KC * H], bf16, kind="ExternalInput")
        dram[f"wa{g}"] = nc.dram_tensor(f"wa{g}", [L, 128, KC * H], wdt, kind="ExternalInput")
        dram[f"f1{g}"] = nc.dram_tensor(f"f1{g}", [L, 128, KC * FH], wdt, kind="ExternalInput")
        dram[f"f2{g}"] = nc.dram_tensor(f"f2{g}", [L, 128, FKC * H], wdt, kind="ExternalInput")
        dram[f"par{g}"] = nc.dram_tensor(f"par{g}", [128, 128], f32, kind="ExternalInput")
        dram[f"out{g}"] = nc.dram_tensor(f"out{g}", [128, KC * C], f32, kind="ExternalOutput")
    dram["ones_s"] = nc.dram_tensor("ones_s", [128, 128], bf16, kind="ExternalInput")
    dram["ones_q"] = nc.dram_tensor("ones_q", [128, 2, 128], fp8, kind="ExternalInput")

    with tile.TileContext(nc) as tc:
        with (
            tc.tile_pool(name="sb", bufs=2) as sb,
            tc.tile_pool(name="ps", bufs=4, space="PSUM") as psp,
        ):
            ones_s = sb.tile([128, 128], bf16, tag="ones_s", bufs=1)
            nc.sync.dma_start(ones_s[:], dram["ones_s"].ap())
            ones_q = sb.tile([128, 2, 128], fp8, tag="ones_q", bufs=1)
            nc.sync.dma_start(ones_q[:], dram["ones_q"].ap())

            def contract(ps, w, rhs, m, nk):
                """ps += w-chunks.T @ rhs-chunks over nk k-chunks of 128.
                w: weight tile (layout depends on mode), m: output block index.
                rhs: [128, nk, C] tile slice."""
                if mode == "dr":
                    for k in range(nk // 2):
                        nc.tensor.matmul(
                            ps[:], w[:, k, m, :, :],
                            rhs[:, 2 * k:2 * k + 2, :],
                            start=(k == 0), stop=(k == nk // 2 - 1), perf_mode=DR)
                else:
                    pm = DP if mode == "dp" else None
                    for k in range(nk):
                        nc.tensor.matmul(
                            ps[:], w[:, k, 128 * m:128 * m + 128], rhs[:, k, :],
                            start=(k == 0), stop=(k == nk - 1), perf_mode=pm)

            def emit_ln(g, C, tiles, y, xn, par, gcol, bcol, skip_beta, skip_gamma, x8):
                """y [128,KC,C] bf16 -> xn = LN(y)*g+b bf16; x8 = fp8(xn) if given."""
                # qscale: q_ps = qscale * E[y^2]; sqrt(qscale) folded into Newton.
                ysq_act = _YSQ_ACT and mode != "bf16"
                qscale = 1.0 if (mode == "bf16" or ysq_act) else 8.0
                rq = math.sqrt(qscale)
                for c0, c1 in tiles:
                    Ct = c1 - c0
                    # mean: (2^-9 ones) @ y, accumulated over KC chunks
                    s_ps = psp.tile([128, Ct], f32, tag=f"st{g}", bufs=1)
                    for k in range(KC):
                        nc.tensor.matmul(s_ps[:], ones_s[:], y[:, k, c0:c1],
                                         start=(k == 0), stop=(k == KC - 1))
                    m_bc = sb.tile([128, Ct], bf16, tag=f"m{g}", bufs=1)
                    nc.vector.tensor_copy(m_bc[:], s_ps[:])
                    msq = sb.tile([128, Ct], bf16, tag=f"msq{g}", bufs=1)
                    nc.gpsimd.tensor_mul(msq[:], m_bc[:], m_bc[:])
                    # E[y^2]: ysq = y*y (Pool), q = scaled-ones @ ysq
                    ysq = sb.tile([128, KC, Ct], fp8 if mode != "bf16" else bf16,
                                  tag=f"ysq{g}", bufs=1)
                    if ysq_act:
                        for k in range(KC):
                            nc.scalar.activation(ysq[:, k, :], y[:, k, c0:c1],
                                                 AF.Square, scale=SQ_SCALE)
                    else:
                        for k in range(KC):
                            nc.gpsimd.tensor_mul(ysq[:, k, :], y[:, k, c0:c1],
                                                 y[:, k, c0:c1])
                    u1s = []
                    def emit_subs():
                        for m in range(KC):
                            u1 = sb.tile([128, Ct], bf16, tag=f"u1{g}", bufs=4)
                            nc.vector.tensor_sub(u1[:], y[:, m, c0:c1], m_bc[:])
                            u1s.append(u1)
                    if _SUBS_EARLY:
                        emit_subs()
                    q_ps = psp.tile([128, Ct], f32, tag=f"st{g}", bufs=1)
                    if mode == "dr":
                        for k in range(KC // 2):
                            nc.tensor.matmul(q_ps[:], ones_q[:],
                                             ysq[:, 2 * k:2 * k + 2, :],
                                             start=(k == 0), stop=(k == KC // 2 - 1),
                                             perf_mode=DR)
                    else:
                        ones_for_q = ones_s[:] if mode == "bf16" else ones_q[:, 0, :]
                        qpm = DP if mode == "dp" else None
                        for k in range(KC):
                            nc.tensor.matmul(q_ps[:], ones_for_q, ysq[:, k, :],
                                             start=(k == 0), stop=(k == KC - 1),
                                             perf_mode=qpm)
                    msqe = sb.tile([128, Ct], bf16, tag=f"msqe{g}", bufs=1)
                    nc.vector.tensor_scalar(msqe[:], msq[:], qscale, qscale * LN_EPS,
                                            ALU.mult, ALU.subtract)
                    z = sb.tile([128, Ct], f32, tag=f"z{g}", bufs=1)
                    nc.vector.tensor_sub(z[:], q_ps[:], msqe[:])
                    # rsqrt(z/qscale): bit-hack (DVE) + 1 Newton step (GPSIMD),
                    # sqrt(qscale) folded into the Newton constants.
                    ti = sb.tile([128, Ct], i32, tag=f"ch{g}", bufs=2)
                    nc.vector.tensor_scalar(ti[:], z[:].bitcast(i32), 1, None,
                                            ALU.arith_shift_right)
                    r0 = sb.tile([128, Ct], f32, tag=f"r0{g}", bufs=1)
                    nc.vector.tensor_scalar(r0[:].bitcast(i32), ti[:], -1, MAGIC,
                                            ALU.mult, ALU.add)
                    u = sb.tile([128, Ct], f32, tag=f"ch{g}", bufs=2)
                    nc.gpsimd.tensor_mul(u[:], r0[:], r0[:])
                    v = sb.tile([128, Ct], f32, tag=f"ch{g}", bufs=2)
                    nc.gpsimd.tensor_mul(v[:], u[:], z[:])
                    w = sb.tile([128, Ct], f32, tag=f"ch{g}", bufs=2)
                    nc.vector.tensor_scalar(w[:], v[:], -0.5 * rq, 1.5 * rq,
                                            ALU.mult, ALU.add)
                    if not _SUBS_EARLY:
                        emit_subs()
                    rstd = sb.tile([128, Ct], bf16, tag=f"rstd{g}", bufs=1)
                    nc.gpsimd.tensor_mul(rstd[:], w[:], r0[:])
                    for m in range(KC):
                        u1 = u1s[m]
                        if skip_gamma and skip_beta:
                            nc.vector.tensor_mul(xn[:, m, c0:c1], u1[:], rstd[:])
                            if x8 is not None:
                                if _X8_DVE:
                                    nc.vector.tensor_copy(x8[:, m, c0:c1], xn[:, m, c0:c1])
                                elif _X8_SPLIT and m >= KC // 2:
                                    nc.vector.tensor_mul(x8[:, m, c0:c1], u1[:], rstd[:])
                                else:
                                    nc.gpsimd.tensor_mul(x8[:, m, c0:c1], u1[:], rstd[:])
                        elif skip_beta:
                            nc.vector.scalar_tensor_tensor(
                                xn[:, m, c0:c1], u1[:], par[:, gcol + m:gcol + m + 1],
                                rstd[:], ALU.mult, ALU.mult)
                            if x8 is not None:
                                nc.scalar.activation(x8[:, m, c0:c1], xn[:, m, c0:c1],
                                                     AF.Copy)
                        else:
                            u2 = sb.tile([128, Ct], bf16, tag=f"u2{g}", bufs=1)
                            nc.vector.scalar_tensor_tensor(
                                u2[:], u1[:], par[:, gcol + m:gcol + m + 1],
                                rstd[:], ALU.mult, ALU.mult)
                            nc.vector.tensor_scalar(
                                xn[:, m, c0:c1], u2[:], par[:, bcol + m:bcol + m + 1],
                                None, ALU.add)
                            if x8 is not None:
                                nc.scalar.activation(x8[:, m, c0:c1], xn[:, m, c0:c1],
                                                     AF.Copy)

            def step_init(st):
                g, C = st["g"], st["C"]
                st["src"] = sb.tile([128, KC, C], bf16, tag=f"src{g}", bufs=_IN_BUFS, name=f"src{g}")
                nc.sync.dma_start(st["src"][:], dram[f"src{g}"].ap())
                st["par"] = sb.tile([128, 128], f32, tag=f"par{g}", bufs=1, name=f"parw{g}")
                nc.sync.dma_start(st["par"][:], dram[f"par{g}"].ap())
                st["io"] = sb.tile([128, KC * H], bf16, tag=f"io{g}", bufs=1, name=f"iow{g}")
                nc.sync.dma_start(st["io"][:], dram[f"io{g}"].ap())
                st["tgt"] = sb.tile([128, KC, C], wdt, tag=f"tgt{g}", bufs=_IN_BUFS, name=f"tgt{g}")
                nc.sync.dma_start(st["tgt"][:], dram[f"tgt{g}"].ap())
                st["op"] = sb.tile([128, KC * H], bf16, tag=f"op{g}", bufs=1, name=f"opw{g}")
                nc.sync.dma_start(st["op"][:], dram[f"op{g}"].ap())

            def step_io(st):
                g, C, par = st["g"], st["C"], st["par"]
                x = sb.tile([128, KC, C], bf16, tag=f"x{g}", bufs=2, name=f"x{g}")
                for c0, c1 in st["tiles"]:
                    for m in range(KC):
                        ps = psp.tile([128, c1 - c0], f32, tag=f"mm{g}", bufs=(_MM_BUFS if g == 0 else 6 - _MM_BUFS) if _MM_ASYM else _MM_BUFS, name=f"ps{g}")
                        for k in range(KC):
                            nc.tensor.matmul(
                                ps[:], st["io"][:, k * H + 128 * m:k * H + 128 * (m + 1)],
                                st["src"][:, k, c0:c1],
                                start=(k == 0), stop=(k == KC - 1))
                        nc.scalar.activation(x[:, m, c0:c1], ps[:], AF.Identity,
                                             bias=par[:, _P_IPB + m:_P_IPB + m + 1])
                st["x"] = x

            def step_attn(st, l):
                g, C, par, x = st["g"], st["C"], st["par"], st["x"]
                wbufs = 1 if mode == "bf16" else 2
                if mode == "dr":
                    st["wa"] = sb.tile([128, KC // 2, H // 128, 2, 128], wdt, tag=f"wa{g}", bufs=2, name=f"waw{g}")
                    (nc.gpsimd if _DMA_SPLIT else nc.sync).dma_start(st["wa"][:], dram[f"wa{g}"].ap()[l])
                    st["f1w"] = sb.tile([128, KC // 2, FH // 128, 2, 128], wdt, tag=f"f1{g}", bufs=wbufs, name=f"f1w{g}")
                    nc.sync.dma_start(st["f1w"][:], dram[f"f1{g}"].ap()[l])
                    st["f2w"] = sb.tile([128, FKC // 2, H // 128, 2, 128], wdt, tag=f"f2{g}", bufs=wbufs, name=f"f2w{g}")
                    nc.sync.dma_start(st["f2w"][:], dram[f"f2{g}"].ap()[l])
                else:
                    st["wa"] = sb.tile([128, KC, H], wdt, tag=f"wa{g}", bufs=2, name=f"waw{g}")
                    nc.sync.dma_start(st["wa"][:], dram[f"wa{g}"].ap()[l])
                    st["f1w"] = sb.tile([128, KC, FH], wdt, tag=f"f1{g}", bufs=wbufs, name=f"f1w{g}")
                    nc.sync.dma_start(st["f1w"][:], dram[f"f1{g}"].ap()[l])
                    st["f2w"] = sb.tile([128, FKC, H], wdt, tag=f"f2{g}", bufs=wbufs, name=f"f2w{g}")
                    nc.sync.dma_start(st["f2w"][:], dram[f"f2{g}"].ap()[l])
                pb = _P_LAYER + 40 * l
                y = sb.tile([128, KC, C], bf16, tag=f"y{g}", bufs=1, name=f"y{g}")
                for c0, c1 in st["tiles"]:
                    for m in range(KC):
                        ps = psp.tile([128, c1 - c0], f32, tag=f"mm{g}", bufs=(_MM_BUFS if g == 0 else 6 - _MM_BUFS) if _MM_ASYM else _MM_BUFS, name=f"ps{g}")
                        contract(ps, st["wa"], st["tgt"][:, :, c0:c1], m, KC)
                        nc.vector.scalar_tensor_tensor(
                            y[:, m, c0:c1], ps[:],
                            par[:, pb + m:pb + m + 1], x[:, m, c0:c1],
                            ALU.add, ALU.add)
                st["y"] = y

            def step_ln1(st, l):
                g, C = st["g"], st["C"]
                pb = _P_LAYER + 40 * l
                xn = sb.tile([128, KC, C], bf16, tag=f"x{g}", bufs=2, name=f"x{g}")
                if mode == "bf16":
                    x8 = None
                else:
                    x8 = sb.tile([128, KC, C], fp8, tag=f"x8{g}", bufs=1, name=f"x8{g}")
                emit_ln(g, C, st["tiles"], st["y"], xn, st["par"],
                        pb + 24, pb + 28, skip_b[g][0], skip_b[g][2], x8)
                st["x"] = xn
                st["x8"] = x8 if x8 is not None else xn

            def step_f1(st, l):
                g, C, par = st["g"], st["C"], st["par"]
                pb = _P_LAYER + 40 * l
                hh = sb.tile([128, FKC, C], fp8 if mode != "bf16" else bf16,
                             tag=f"h{g}", bufs=_H_BUFS, name=f"h{g}")
                for c0, c1 in st["tiles"]:
                    for m in range(FKC):
                        ps = psp.tile([128, c1 - c0], f32, tag=f"mm{g}", bufs=(_MM_BUFS if g == 0 else 6 - _MM_BUFS) if _MM_ASYM else _MM_BUFS, name=f"ps{g}")
                        contract(ps, st["f1w"], st["x8"][:, :, c0:c1], m, KC)
                        nc.scalar.activation(hh[:, m, c0:c1], ps[:], AF.Gelu,
                                             bias=par[:, pb + 4 + m:pb + 4 + m + 1])
                st["h"] = hh

            def step_f2(st, l):
                g, C, par = st["g"], st["C"], st["par"]
                pb = _P_LAYER + 40 * l
                y2 = sb.tile([128, KC, C], bf16, tag=f"y{g}", bufs=1, name=f"y{g}")
                for c0, c1 in st["tiles"]:
                    for m in range(KC):
                        ps = psp.tile([128, c1 - c0], f32, tag=f"mm{g}", bufs=(_MM_BUFS if g == 0 else 6 - _MM_BUFS) if _MM_ASYM else _MM_BUFS, name=f"ps{g}")
                        contract(ps, st["f2w"], st["h"][:, :, c0:c1], m, FKC)
                        if _DRAIN_ACT:
                            t2 = sb.tile([128, c1 - c0], bf16, tag=f"t2{g}", bufs=2, name=f"t2{g}")
                            nc.scalar.activation(t2[:], ps[:], AF.Identity,
                                                 bias=par[:, pb + 20 + m:pb + 20 + m + 1])
                            nc.vector.tensor_add(y2[:, m, c0:c1], t2[:],
                                                 st["x"][:, m, c0:c1])
                        else:
                            nc.vector.scalar_tensor_tensor(
                                y2[:, m, c0:c1], ps[:],
                                par[:, pb + 20 + m:pb + 20 + m + 1],
                                st["x"][:, m, c0:c1], ALU.add, ALU.add)
                st["y"] = y2

            def step_ln2(st, l):
                g, C = st["g"], st["C"]
                pb = _P_LAYER + 40 * l
                xn2 = sb.tile([128, KC, C], bf16, tag=f"x{g}", bufs=2, name=f"x{g}")
                emit_ln(g, C, st["tiles"], st["y"], xn2, st["par"],
                        pb + 32, pb + 36, skip_b[g][1], skip_b[g][3], None)
                st["x"] = xn2

            def step_op(st):
                g, C, par = st["g"], st["C"], st["par"]
                for c0, c1 in st["tiles"]:
                    for m in range(KC):
                        ps = psp.tile([128, c1 - c0], f32, tag=f"mm{g}", bufs=(_MM_BUFS if g == 0 else 6 - _MM_BUFS) if _MM_ASYM else _MM_BUFS, name=f"ps{g}")
                        for k in range(KC):
                            nc.tensor.matmul(
                                ps[:],
                                st["op"][:, k * H + 128 * m:k * H + 128 * (m + 1)],
                                st["x"][:, k, c0:c1],
                                start=(k == 0), stop=(k == KC - 1))
                        ot = sb.tile([128, c1 - c0], f32, tag=f"o{g}", bufs=_IN_BUFS, name=f"ot{g}")
                        nc.scalar.activation(ot[:], ps[:], AF.Identity,
                                             bias=par[:, _P_OPB + m:_P_OPB + m + 1])
                        nc.sync.dma_start(
                            dram[f"out{g}"].ap()[:, m * C + c0:m * C + c1], ot[:])

            for _rep in range(repeat):
                sts = [{"g": g, "C": C, "tiles": _ntiles(C)}
                       for g, C in ((0, C0), (1, C1))]

                def steps_for(st):
                    seq = [lambda st=st: (step_init(st), step_io(st))]
                    for l in range(L):
                        seq.append(lambda st=st, l=l: step_attn(st, l))
                        seq.append(lambda st=st, l=l: step_ln1(st, l))
                        seq.append(lambda st=st, l=l: step_f1(st, l))
                        seq.append(lambda st=st, l=l: step_f2(st, l))
                        seq.append(lambda st=st, l=l: step_ln2(st, l))
                    seq.append(lambda st=st: step_op(st))
                    return seq

                s0, s1 = steps_for(sts[0]), steps_for(sts[1])
                for i in range(len(s0) + _SKEW):
                    if i < len(s0):
                        s0[i]()
                    if 0 <= i - _SKEW < len(s1):
                        s1[i - _SKEW]()

    nc.compile()
    return nc


_CACHE = {}


def _get_program(C0, C1, skip_b, repeat=1):
    key = (C0, C1, skip_b, repeat, _MM_MODE, _SKEW, _CAST_ACT,
           _YSQ_ACT, _X8_DVE, _H_BUFS, _DMA_SPLIT, _SUBS_EARLY, _IN_BUFS,
           _DRAIN_ACT, _MM_BUFS, _MM_ASYM, _X8_SPLIT)
    if key not in _CACHE:
        _CACHE[key] = _build_program(C0, C1, skip_b, repeat)
    return _CACHE[key]


def _np_wdt():
    return NP_BF16 if _MM_MODE == "bf16" else NP_FP8


def _prep_gen_weights(i, g_ipw, g_ipb, g_qkv_w, g_qkv_b, g_ao_w, g_ao_b,
                      g_ln1g, g_ln1b, g_ln2g, g_ln2b, g_f1w, g_f1b, g_f2w,
                      g_f2b, g_opw, g_opb, g_rw):
    np_wdt = _np_wdt()
    wa, ba = [], []
    for l in range(L):
        _wq, _wk, wv = np.split(g_qkv_w[i, l], 3, axis=0)
        _bq, _bk, bv = np.split(g_qkv_b[i, l], 3)
        wa.append((g_ao_w[i, l] @ wv).T)                 # [K=H, M=H]
        ba.append(g_ao_b[i, l] + bv @ g_ao_w[i, l].T)
    rw = float(g_rw[i])
    io = _sb_pack(g_ipw[i].T).astype(NP_BF16)
    op = _sb_pack((1.0 - rw) * g_opw[i].T).astype(NP_BF16)
    packw = _sb_pack_dr if _MM_MODE == "dr" else _sb_pack
    waP = np.stack([packw(wa[l]).astype(np_wdt) for l in range(L)])
    f1P = np.stack([packw(g_f1w[i, l].T).astype(np_wdt) for l in range(L)])
    f2P = np.stack([packw(g_f2w[i, l].T).astype(np_wdt) for l in range(L)])

    par = np.zeros((128, 128), np.float32)
    par[:, _P_IPB:_P_IPB + KC] = _pack_pcol(g_ipb[i])
    for l in range(L):
        pb = _P_LAYER + 40 * l
        par[:, pb:pb + 4] = _pack_pcol(ba[l])
        par[:, pb + 4:pb + 20] = _pack_pcol(g_f1b[i, l])
        par[:, pb + 20:pb + 24] = _pack_pcol(g_f2b[i, l])
        par[:, pb + 24:pb + 28] = _pack_pcol(g_ln1g[i, l])
        par[:, pb + 28:pb + 32] = _pack_pcol(g_ln1b[i, l])
        par[:, pb + 32:pb + 36] = _pack_pcol(g_ln2g[i, l])
        par[:, pb + 36:pb + 40] = _pack_pcol(g_ln2b[i, l])
    par[:, _P_OPB:_P_OPB + KC] = _pack_pcol((1.0 - rw) * g_opb[i])

    skip = (bool(np.all(g_ln1b[i] == 0.0)), bool(np.all(g_ln2b[i] == 0.0)),
            bool(np.all(g_ln1g[i] == 1.0)), bool(np.all(g_ln2g[i] == 1.0)))
    return {"io": io, "op": op, "wa": waP, "f1": f1P, "f2": f2P, "par": par}, skip, rw


def _prepare(inputs, repeat=1):
    """Host-side prep. Returns (nc, in_maps, assemble) where assemble(results)
    builds the final outputs."""
    image = np.asarray(inputs["image_features"], np.float32)
    text = np.asarray(inputs["text_features"], np.float32)
    mt = np.asarray(inputs["missing_type"])

    idx1 = np.nonzero(mt == 1)[0]      # gen0 (img -> text) fills text
    idx2 = np.nonzero(mt == 2)[0]      # gen1 (text -> img) fills img
    idx3 = np.nonzero(mt == 3)[0]

    gw = {k: np.asarray(v) for k, v in inputs.items() if k.startswith("g_")}
    w0, skip0, rw0 = _prep_gen_weights(0, **gw)
    w1, skip1, rw1 = _prep_gen_weights(1, **gw)

    # prior MLP on host (tiny)
    pe = np.asarray(inputs["prior_emb"], np.float64)
    t = pe @ np.asarray(inputs["prior_w1"], np.float64).T + np.asarray(inputs["prior_b1"], np.float64)
    t = 0.5 * t * (1.0 + np.vectorize(math.erf)(t / math.sqrt(2.0)))
    prior = (t @ np.asarray(inputs["prior_w2"], np.float64).T
             + np.asarray(inputs["prior_b2"], np.float64)).astype(np.float32)
    p_img, p_text = prior[0, :H], prior[0, H:]

    imgT = np.ascontiguousarray(image.T)
    textT = np.ascontiguousarray(text.T)
    np_wdt = _np_wdt()

    def shard_cols(Tsrc, Ttgt, idx):
        n_pc = max(1, -(-len(idx) // N_CORES))
        C = max(256, -(-n_pc // 32) * 32)
        pad = np.zeros(N_CORES * C, np.int64)
        pad[:len(idx)] = idx
        pad = pad.reshape(N_CORES, C)
        return C, [_sb_pack(Tsrc[:, pad[c]]).astype(NP_BF16) for c in range(N_CORES)], \
            [_sb_pack(Ttgt[:, pad[c]]).astype(np_wdt) for c in range(N_CORES)]

    C0, src0, tgt0 = shard_cols(imgT, textT, idx1)
    C1, src1, tgt1 = shard_cols(textT, imgT, idx2)

    nc = _get_program(C0, C1, (skip0, skip1), repeat)

    ones_s = np.full((128, 128), ONES_S, NP_BF16)
    ones_q = np.full((128, 2, 128), ONES_Q, NP_FP8)
    in_maps = []
    for c in range(N_CORES):
        in_maps.append({
            "src0": src0[c], "tgt0": tgt0[c], "src1": src1[c], "tgt1": tgt1[c],
            "io0": w0["io"], "op0": w0["op"], "wa0": w0["wa"], "f10": w0["f1"],
            "f20": w0["f2"], "par0": w0["par"],
            "io1": w1["io"], "op1": w1["op"], "wa1": w1["wa"], "f11": w1["f1"],
            "f21": w1["f2"], "par1": w1["par"],
            "ones_s": ones_s, "ones_q": ones_q,
        })

    def assemble(results):
        def gather_out(name, C, idx, rw, full):
            cols = [results[c][name].reshape(128, KC, C).transpose(1, 0, 2).reshape(H, C)
                    for c in range(N_CORES)]
            allc = np.concatenate(cols, axis=1)[:, :len(idx)]
            return rw * full[idx] + allc.T

        enhanced_text = text.copy()
        if len(idx1):
            enhanced_text[idx1] = gather_out("out0", C0, idx1, rw0, text)
        enhanced_img = image.copy()
        if len(idx2):
            enhanced_img[idx2] = gather_out("out1", C1, idx2, rw1, image)
        if len(idx3):
            enhanced_img[idx3] = p_img
            enhanced_text[idx3] = p_text
        return enhanced_img, enhanced_text

    return nc, in_maps, assemble


def kernel(**inputs):
    nc, in_maps, assemble = _prepare(inputs)
    res = run_bass_kernel_spmd(nc, in_maps, list(range(N_CORES)))
    return assemble(res.results)
